# revision 23
# baseline (speedup 1.0000x reference)
"""Trainium2 Bass kernel for nn_Bottleneck_57561151701110, v3.

v3 changes vs v2 (199us):
  - conv1/conv2 composed with the band selector: per-band matmuls with
    tile_position col-tiling write x1/x2 directly in band-partition layout
    into one shared PSUM tile. Kills x12s, the selector matmuls, and 23
    evac ops.
  - ACT op count halved; single-op Lrelu (HW LEAKY_RELU) for both
    epilogues with fused per-partition bias.
  - 1/Z via ACT Ln + Exp(-x) instead of the 6us DVE reciprocal.
  - DMA issues spread across sync + gpsimd queues (scalar keeps ACT);
    DMA count cut ~2x via multi-band APs (sam stores, smc loads, out).
  - fr relu on DVE tensor_scalar (4x mode), hp/exp on ACT.
"""

import os
import sys

for _p in ("/opt/trn_rl_repo", os.path.expanduser("~/.axon_site/_ro/trn_rl_repo")):
    if os.path.isdir(_p) and _p not in sys.path:
        sys.path.insert(0, _p)

from contextlib import ExitStack

import numpy as np

import concourse.bass as bass
import concourse.bacc as bacc
import concourse.tile as tile
from concourse import mybir
from concourse.bass_utils import run_bass_kernel_spmd

dt = mybir.dt
ALU = mybir.AluOpType
ACTF = mybir.ActivationFunctionType

B, CIN, H, W = 8, 256, 56, 56
NPIX = H * W            # 3136
REL, MID, OUT = 32, 256, 256
SHARE = 8
NB = 4                  # row bands
BH = H // NB            # 14 rows per band
Q = BH * W              # 784 band pixels
HQ = Q // 2             # 392 half-band pixels
BR = BH + 2             # 16 padded band rows
NEG = 0.01
BN_EPS = 1e-5
CCH = 448               # conv3 free chunk (8 rows)
NCH = NPIX // CCH       # 7
OFFS = [(dh, dw) for dh in (-1, 0, 1) for dw in (-1, 0, 1)]

F32, F16 = dt.float32, dt.float16

_CACHE = {}


def _position(h, w):
    loc_w = np.tile(np.linspace(-1.0, 1.0, w, dtype=np.float32)[None, :], (h, 1))
    loc_h = np.tile(np.linspace(-1.0, 1.0, h, dtype=np.float32)[:, None], (1, w))
    return np.stack([loc_w, loc_h], axis=0)


def _host_consts(inp):
    f32, f16 = np.float32, np.float16
    inv_a = (inp["bna_g"] / np.sqrt(inp["bna_v"] + BN_EPS)).astype(f32)
    beta_a = (inp["bna_b"] - inp["bna_m"] * inv_a).astype(f32)
    inv_b = (inp["bnb_g"] / np.sqrt(inp["bnb_v"] + BN_EPS)).astype(f32)
    beta_b = (inp["bnb_b"] - inp["bnb_m"] * inv_b).astype(f32)

    w1c = inp["conv1_w"] * inv_a[:REL, None]          # (32, 256)
    b1 = inp["conv1_b"] * inv_a[:REL] + beta_a[:REL]
    w2c = inp["conv2_w"] * inv_a[:REL, None]
    b2 = inp["conv2_b"] * inv_a[:REL]

    cx1 = w1c.T.astype(f16).copy()   # (256, 32) lhsT
    cx2 = w2c.T.astype(f16).copy()
    c3 = inp["conv3_w"].T.astype(f16).copy()
    co = inp["convo_w"].T.astype(f16).copy()

    w1p = (inp["w1"] * inv_b[:, None]).astype(f32)
    w1a, w1b = w1p[:, :REL], w1p[:, REL:]
    lhsT_w1 = np.zeros((128, 128), f32)
    lhsT_pos = np.zeros((8, 128), f32)
    lhsT_w2 = np.zeros((128, 128), f32)
    for b in range(NB):
        lhsT_w1[32 * b:32 * b + 32, 32 * b:32 * b + 32] = w1a.T
        lhsT_pos[2 * b:2 * b + 2, 32 * b:32 * b + 32] = w1b.T
        lhsT_w2[32 * b:32 * b + 32, 32 * b:32 * b + 32] = inp["w2"].T

    pos = _position(H, W)
    pc = np.einsum("oc,chw->ohw", inp["convp_w"], pos) + inp["convp_b"][:, None, None]
    pcp = np.pad(pc, ((0, 0), (1, 1), (1, 1)))
    posr = np.zeros((8, 9 * Q), f32)
    for k, (dh, dw) in enumerate(OFFS):
        sub = pc - pcp[:, 1 + dh:1 + dh + H, 1 + dw:1 + dw + W]
        v = np.maximum(inv_a[REL:, None, None] * sub + beta_a[REL:, None, None], 0.0)
        vb = v.reshape(2, NB, BH, W)
        for b in range(NB):
            posr[2 * b:2 * b + 2, Q * k:Q * (k + 1)] = vb[:, b].reshape(2, Q)

    vecs = np.zeros((128, 8), f32)
    vecs[:, 0] = np.tile(b1, NB)
    vecs[:, 1] = np.tile(b2, NB)
    vecs[:, 2] = inp["conv3_b"][:128]
    vecs[:, 3] = inp["conv3_b"][128:]
    vecs[:, 4] = np.tile(beta_b, NB)
    vecs[:, 5] = np.tile(inp["w2_b"], NB)
    vecs[:, 6] = inp["convo_b"][:128]
    vecs[:, 7] = inp["convo_b"][128:]

    return {
        "cx1": cx1, "cx2": cx2, "c3": c3, "co": co,
        "lhsT_w1": lhsT_w1.astype(f16), "lhsT_pos": lhsT_pos.astype(f16),
        "lhsT_w2": lhsT_w2.astype(f16),
        "posr": posr.astype(f16), "vecs": vecs,
        "identb": np.eye(128, dtype=f16),
    }


def _build_program():
    nc = bacc.Bacc("TRN2", target_bir_lowering=False, debug=False,
                   enable_asserts=False, num_devices=8)

    def dram_in(name, shape, dtype):
        return nc.dram_tensor(name, list(shape), dtype, kind="ExternalInput").ap()

    xin = dram_in("xin", (CIN, NPIX), F16)
    cx1d = dram_in("cx1", (CIN, REL), F16)
    cx2d = dram_in("cx2", (CIN, REL), F16)
    c3d = dram_in("c3", (CIN, CIN), F16)
    cod = dram_in("co", (CIN, CIN), F16)
    w1d = dram_in("lhsT_w1", (128, 128), F16)
    posd = dram_in("lhsT_pos", (8, 128), F16)
    w2d = dram_in("lhsT_w2", (128, 128), F16)
    posrd = dram_in("posr", (8, 9 * Q), F16)
    vecsd = dram_in("vecs", (128, 8), F32)
    identd = dram_in("identb", (128, 128), F16)

    outd = nc.dram_tensor("out", [CIN, NPIX], F16, kind="ExternalOutput").ap()
    outv = outd.rearrange("c (b h q) -> c b h q", b=NB, h=2, q=HQ)

    x3b = [nc.dram_tensor(f"x3b{b}", [CIN, BR * 58], F16).ap() for b in range(NB)]
    samd = [nc.dram_tensor(f"samd{qp}", [NB * CIN * HQ], F16).ap()
            for qp in range(2)]
    # store view: partitions (b g), free (e q); load view: partitions c, free (b q)
    sams = [samd[qp].rearrange("(b g e q) -> (b g) (e q)", b=NB, g=32, q=HQ)
            for qp in range(2)]
    saml = [samd[qp].rearrange("(b c q) -> c b q", b=NB, q=HQ) for qp in range(2)]

    with tile.TileContext(nc) as tc, ExitStack() as ctx:
        ctx.enter_context(nc.allow_low_precision(reason="fp16 kernel, 2e-2 gate"))
        nc_ = tc.nc

        cpool = ctx.enter_context(tc.tile_pool(name="consts", bufs=1))
        prpool = ctx.enter_context(tc.tile_pool(name="prpool", bufs=9))

        # consts cx1/cx2 first (gps queue), xin quarters on sync, rest after
        cx1t = cpool.tile([128, 2, REL], F16, tag="cx1t")
        cx2t = cpool.tile([128, 2, REL], F16, tag="cx2t")
        c3t = cpool.tile([128, 2, CIN], F16, tag="c3t")
        cot = cpool.tile([128, 2, CIN], F16, tag="cot")
        for kc in range(2):
            ksl = slice(128 * kc, 128 * (kc + 1))
            nc_.gpsimd.dma_start(cx1t[:, kc, :], cx1d[ksl, :])
            nc_.gpsimd.dma_start(cx2t[:, kc, :], cx2d[ksl, :])
        xt = cpool.tile([128, 2, NPIX], F16, tag="xt")
        bnds = list(range(0, NPIX, 512)) + [NPIX]
        for j in range(len(bnds) - 1):
            lo, hi = bnds[j], bnds[j + 1]
            for t in range(2):
                eng = nc.sync if t == 0 else nc.scalar
                eng.dma_start(xt[:, t, lo:hi],
                              xin[128 * t:128 * (t + 1), lo:hi])
        prts = [None] * 9
        for k in range(4):
            prt = prpool.tile([8, Q], F16, tag="prt", name="prt")
            nc.sync.dma_start(prt[:], posrd[:, Q * k:Q * (k + 1)])
            prts[k] = prt
        for kc in range(2):
            ksl = slice(128 * kc, 128 * (kc + 1))
            nc_.gpsimd.dma_start(c3t[:, kc, :], c3d[ksl, :])
            nc_.gpsimd.dma_start(cot[:, kc, :], cod[ksl, :])
        w1t = cpool.tile([128, 128], F16, tag="w1t")
        nc_.gpsimd.dma_start(w1t[:], w1d[:])
        post = cpool.tile([8, 128], F16, tag="post")
        nc_.gpsimd.dma_start(post[:], posd[:])
        w2t = cpool.tile([128, 128], F16, tag="w2t")
        nc_.gpsimd.dma_start(w2t[:], w2d[:])
        vecst = cpool.tile([128, 8], F32, tag="vecst")
        nc_.gpsimd.dma_start(vecst[:], vecsd[:])
        identt = cpool.tile([128, 128], F16, tag="identt")
        nc_.gpsimd.dma_start(identt[:], identd[:])
        zlhs = cpool.tile([128, REL], F16, tag="zlhs")
        nc_.gpsimd.memset(zlhs[:], 0.0)
        zt = cpool.tile([128, 58], F16, tag="zt")
        nc_.gpsimd.memset(zt[:], 0.0)
        for t in range(2):
            tsl = slice(128 * t, 128 * (t + 1))
            nc_.gpsimd.dma_start(x3b[0][tsl, 0:58], zt[:])
            nc_.gpsimd.dma_start(x3b[NB - 1][tsl, 15 * 58:16 * 58], zt[:])

        # band tiles
        bpool = ctx.enter_context(tc.tile_pool(name="bpool", bufs=1))
        x1b = bpool.tile([128, Q], F16, tag="x1b")
        x2bA = bpool.tile([128, BR, 58], F16, tag="x2bA")
        x2bB = bpool.tile([128, BR, 58], F16, tag="x2bB")
        nc_.gpsimd.memset(x2bA[:], 0.0)
        xgpool = ctx.enter_context(tc.tile_pool(name="xgpool", bufs=1))
        xgA = xgpool.tile([128, SHARE, BR, 58], F16, tag="xgA")
        xgB = xgpool.tile([128, SHARE, BR, 58], F16, tag="xgB")

        # ---- phase A: composed conv1/conv2 into band layout + conv3
        ascope = ExitStack()
        ppA = ascope.enter_context(tc.tile_pool(name="ppA", bufs=1, space="PSUM"))

        psA = ppA.tile([128, 2048], F32, tag="psA")
        for b in range(NB):
            psl = slice(32 * b, 32 * b + 32)
            tp = (0, 32 * b)
            # x1: band pixels Q*b..Q*b+784 -> psA[32b.., 0:784]
            for j, (o0, n) in enumerate(((0, 512), (512, 272))):
                for kc in range(2):
                    nc_.tensor.matmul(psA[psl, o0:o0 + n], cx1t[:, kc, :],
                                      xt[:, kc, Q * b + o0:Q * b + o0 + n],
                                      start=(kc == 0), stop=(kc == 1),
                                      tile_position=tp)
            # x2: rows 14b-1..14b+15 -> psA[32b.., 1024 + 56*r]
            rlo = 1 if b == 0 else 0
            rhi = 15 if b == NB - 1 else 16
            p0 = (BH * b - 1 + rlo) * W
            c0 = 1024 + rlo * W
            n_tot = (rhi - rlo) * W
            pieces = []
            cur = c0
            while cur < c0 + n_tot:
                bank_end = (cur // 512 + 1) * 512
                n = min(bank_end, c0 + n_tot) - cur
                pieces.append((cur - c0, n))
                cur += n
            for (o0, n) in pieces:
                for kc in range(2):
                    nc_.tensor.matmul(psA[psl, c0 + o0:c0 + o0 + n],
                                      cx2t[:, kc, :],
                                      xt[:, kc, p0 + o0:p0 + o0 + n],
                                      start=(kc == 0), stop=(kc == 1),
                                      tile_position=tp)
            # zero the missing halo row at image boundary
            if b == 0:
                nc_.tensor.matmul(psA[psl, 1024:1024 + W], zlhs[0:128, :],
                                  xt[:, 0, 0:W], start=True, stop=True,
                                  tile_position=tp)
            if b == NB - 1:
                nc_.tensor.matmul(psA[psl, 1024 + 15 * W:1024 + 16 * W],
                                  zlhs[0:128, :], xt[:, 0, 0:W],
                                  start=True, stop=True, tile_position=tp)

        # evac band tiles from psA (after all A' matmuls)
        nc_.vector.tensor_scalar(x1b[:], psA[:, 0:Q], vecst[:, 0:1], None,
                                 op0=ALU.add)
        nc_.vector.tensor_scalar(
            x2bA[:, :, 1:57],
            psA[:, 1024:1024 + BR * W].rearrange("p (r w) -> p r w", w=W),
            vecst[:, 1:2], None, op0=ALU.add)
        # image-boundary halo rows of x2bA must stay exactly zero
        nc_.gpsimd.memset(x2bA[0:32, 0, :], 0.0)
        nc_.gpsimd.memset(x2bA[96:128, 15, :], 0.0)
        nfl2 = BR * 58
        av2 = x2bA[:].rearrange("p r w -> p (r w)")
        bv2 = x2bB[:].rearrange("p r w -> p (r w)")
        nc.sync.dma_start(bv2[:, 1:nfl2], av2[:, 0:nfl2 - 1])

        # conv3 -> banded DRAM scratch
        for c in range(NCH):
            for t in range(2):
                ps3 = ppB.tile([128, CCH], F32, tag="ps3")
                nc_.tensor.matmul(ps3[:], c3t[:, 0, 128 * t:128 * (t + 1)],
                                  xt[:, 0, CCH * c:CCH * (c + 1)],
                                  start=True, stop=False)
                nc_.tensor.matmul(ps3[:], c3t[:, 1, 128 * t:128 * (t + 1)],
                                  xt[:, 1, CCH * c:CCH * (c + 1)],
                                  start=False, stop=True)
                x3s = x3s_pp[t][c % 3]
                if t == 0:
                    nc_.vector.tensor_scalar(
                        x3s[:, :, 1:57],
                        ps3[:].rearrange("p (r w) -> p r w", w=W),
                        vecst[:, 2 + t:3 + t], None, op0=ALU.add)
                else:
                    nc.scalar.activation(
                        x3s[:, :, 1:57],
                        ps3[:].rearrange("p (r w) -> p r w", w=W),
                        ACTF.Identity, bias=vecst[:, 2 + t:3 + t])
                for b in range(NB):
                    lo = max(8 * c, BH * b - 1)
                    hi = min(8 * (c + 1), BH * b + BH + 1)
                    if lo >= hi:
                        continue
                    nc.sync.dma_start(
                        x3b[b][128 * t:128 * (t + 1),
                               (lo - (BH * b - 1)) * 58:(hi - (BH * b - 1)) * 58],
                        x3s[:, lo - 8 * c:hi - 8 * c, :])

        # xg loads (after conv3 stores; deps via DRAM tensors)
        for b in range(NB):
            psl = slice(32 * b, 32 * b + 32)
            eng = nc.sync if b % 2 == 0 else nc.scalar
            eng.dma_start(xgA[psl].rearrange("p s r w -> p (s r w)"),
                          x3b[b][:].rearrange("(g s) f -> g (s f)", s=SHARE))
        nflat = SHARE * BR * 58
        avg = xgA[:].rearrange("p s r w -> p (s r w)")
        bvg = xgB[:].rearrange("p s r w -> p (s r w)")
        nc.sync.dma_start(bvg[:, 1:nflat], avg[:, 0:nflat - 1])

        # ---- phase D: logits per k
        ascope.close()

        epool = ctx.enter_context(tc.tile_pool(name="epool", bufs=9))
        rzpool = ctx.enter_context(tc.tile_pool(name="rzpool", bufs=1))
        pkpool = ctx.enter_context(tc.tile_pool(name="pkpool", bufs=11))
        fpool = ctx.enter_context(tc.tile_pool(name="fpool", bufs=2))
        hppool = ctx.enter_context(tc.tile_pool(name="hppool", bufs=2))

        dscope = ExitStack()
        fpool = dscope.enter_context(tc.tile_pool(name="fpool", bufs=2))
        hppool = dscope.enter_context(tc.tile_pool(name="hppool", bufs=2))
        ppD = dscope.enter_context(tc.tile_pool(name="ppD", bufs=4, space="PSUM"))

        wsl = [slice(0, 512), slice(512, Q)]

        def mk_pk(k, qp):
            dh, dw = OFFS[k]
            if dw == 0:
                src_, co = xgB, 2
            else:
                src_, co = (xgA, 0) if dw == -1 else (xgA, 2)
            r0 = 1 + dh + 7 * qp
            pk = pkpool.tile([128, SHARE, 7, W], F16, tag="pk", name="pk")
            nc_.vector.tensor_tensor(
                pk[:], src_[:, :, r0:r0 + 7, co:co + W],
                ek[k][:].rearrange("p (r w) -> p r w", w=W)
                [:, 7 * qp:7 * qp + 7, :].unsqueeze(1)
                .broadcast_to((128, SHARE, 7, W)),
                ALU.mult)
            return pk

        pks = {}
        ek = []
        for k, (dh, dw) in enumerate(OFFS):
            co2 = 0 if dw == -1 else 2
            x2src = x2bB if dw == 0 else x2bA
            prt = prts[k]
            fs = fpool.tile([128, BH, W], F16, tag="fs")
            nc_.vector.tensor_tensor(
                fs[:], x1b[:].rearrange("p (r w) -> p r w", w=W),
                x2src[:, 1 + dh:1 + dh + BH, co2:co2 + W], ALU.subtract)
            fr = fpool.tile([128, Q], F16, tag="fr")
            nc_.vector.tensor_scalar(fr[:], fs[:].rearrange("p r w -> p (r w)"),
                                     0.0, None, op0=ALU.max)
            hps = ppD.tile([128, 1024], F32, tag="hw")
            for s in wsl:
                nc_.tensor.matmul(hps[:, s], w1t[:], fr[:, s],
                                  start=True, stop=False)
                nc_.tensor.matmul(hps[:, s], post[:], prt[:, s],
                                  start=False, stop=True)
            hp = hppool.tile([128, Q], F16, tag="hp")
            nc.scalar.activation(hp[:], hps[:, 0:Q], ACTF.Relu,
                                 bias=vecst[:, 4:5])
            wps = ppD.tile([128, 1024], F32, tag="hw")
            for s in wsl:
                nc_.tensor.matmul(wps[:, s], w2t[:], hp[:, s],
                                  start=True, stop=True)
            e = epool.tile([128, Q], F16, tag="e")
            nc.scalar.activation(e[:], wps[:, 0:Q], ACTF.Exp,
                                 bias=vecst[:, 5:6])
            if k == 0:
                nc_.vector.tensor_copy(zacc[:], e[:])
            else:
                nc_.vector.tensor_tensor(zacc[:], zacc[:], e[:], ALU.add)
            ek.append(e)
            if k >= 2:
                pks[(k - 2, 0)] = mk_pk(k - 2, 0)
        zl = rzpool.tile([128, Q], F16, tag="zl")
        nc.scalar.activation(zl[:], zps[:, 0:Q], ACTF.Ln)
        rz16 = rzpool.tile([128, Q], F16, tag="rz16")
        nc.scalar.activation(rz16[:], zl[:], ACTF.Exp, scale=-1.0)
        dscope.close()

        # ---- phase F: aggregation + conv_out
        samp = ctx.enter_context(tc.tile_pool(name="samp", bufs=1, space="PSUM"))
        lspool = ctx.enter_context(tc.tile_pool(name="lspool", bufs=2))
        smcpool = ctx.enter_context(tc.tile_pool(name="smcpool", bufs=4))
        lopool = ctx.enter_context(tc.tile_pool(name="lopool", bufs=2))
        ostpool = ctx.enter_context(tc.tile_pool(name="ostpool", bufs=2))
        ppo = ctx.enter_context(tc.tile_pool(name="ppo", bufs=2, space="PSUM"))

        xtv = xt[:].rearrange("p t (b h q) -> p t b h q", b=NB, h=2)

        def round_accum(qp, sq):
            sam = samp.tile([128, 2048], F32, tag="sam", name="sam")
            for k in range(9):
                for j in range(4):
                    nc_.tensor.matmul(sam[:, 512 * j:512 * j + HQ],
                                      identt[:], pks[(k, qp)][:, 4 * sq + j],
                                      start=(k == 0), stop=(k == 8))
            samv = sam[:].rearrange("p (a j) -> p a j", j=512)[:, :, 0:HQ]
            ls = lspool.tile([128, 4, HQ], F16, tag="ls", name="ls")
            nc.scalar.activation(ls[:], samv, ACTF.Lrelu, alpha=NEG)
            return ls

        def round_finish(qp, sq, ls):
            sq_ = lspool.tile([128, 4, HQ], F16, tag="sq", name="sq_")
            nc_.vector.tensor_tensor(
                sq_[:], ls[:],
                rz16[:, HQ * qp:HQ * (qp + 1)].unsqueeze(1)
                .broadcast_to((128, 4, HQ)),
                ALU.mult)
            nc.sync.dma_start(
                sams[qp][:, 4 * sq * HQ:4 * (sq + 1) * HQ]
                .rearrange("p (e q) -> p e q", e=4),
                sq_[:])

        def convo_loads(qp, bp=None):
            smc = []
            for t in range(2):
                s_ = smcpool.tile([128, 2, HQ], F16, tag="smc", name="smc")
                nc.sync.dma_start(
                    s_[:], saml[qp][128 * t:128 * (t + 1),
                                    2 * bp:2 * bp + 2, :])
                smc.append(s_)
            return smc

        def convo_bp(qp, smc, bp):
            if True:
                lro = lopool.tile([128, 2, 2, HQ], F16, tag="lro", name="lro")
                for o in range(2):
                    pso = ppo.tile([128, 1024], F32, tag="pso", name="pso")
                    for bi in range(2):
                        for kc in range(2):
                            nc_.tensor.matmul(
                                pso[:, 512 * bi:512 * bi + HQ],
                                cot[:, kc, 128 * o:128 * (o + 1)],
                                smc[kc][:, bi, :],
                                start=(kc == 0), stop=(kc == 1))
                    nc.scalar.activation(
                        lro[:, o, :, :],
                        pso[:].rearrange("p (a j) -> p a j", j=512)[:, :, 0:HQ],
                        ACTF.Lrelu, bias=vecst[:, 6 + o:7 + o], alpha=NEG)
                ost = ostpool.tile([128, 2, 2, HQ], F16, tag="ost", name="ost")
                for o in range(2):
                    nc_.vector.tensor_tensor(
                        ost[:, o], lro[:, o],
                        xtv[:, o, 2 * bp:2 * bp + 2, qp, :], ALU.add)
                    nc.sync.dma_start(
                        outv[128 * o:128 * (o + 1), 2 * bp:2 * bp + 2, qp, :],
                        ost[:, o])

        for k in (7, 8):
            pks[(k, 0)] = mk_pk(k, 0)
        ls00 = round_accum(0, 0)
        for k in range(3):
            pks[(k, 1)] = mk_pk(k, 1)
        round_finish(0, 0, ls00)
        ls01 = round_accum(0, 1)
        for k in range(3, 9):
            pks[(k, 1)] = mk_pk(k, 1)
        round_finish(0, 1, ls01)
        smc00 = convo_loads(0, 0)
        smc01 = convo_loads(0, 1)
        ls10 = round_accum(1, 0)
        round_finish(1, 0, ls10)
        convo_bp(0, smc00, 0)
        ls11 = round_accum(1, 1)
        round_finish(1, 1, ls11)
        convo_bp(0, smc01, 1)
        smc10 = convo_loads(1, 0)
        smc11 = convo_loads(1, 1)
        convo_bp(1, smc10, 0)
        convo_bp(1, smc11, 1)

    nc.compile()
    return nc


def _get_program():
    if "nc" not in _CACHE:
        _CACHE["nc"] = _build_program()
    return _CACHE["nc"]


def _in_maps(inputs):
    consts = _host_consts(inputs)
    x = np.asarray(inputs["x"]).reshape(B, CIN, NPIX).astype(np.float16)
    in_maps = []
    for b in range(B):
        m = dict(consts)
        m["xin"] = x[b]
        in_maps.append(m)
    return in_maps


def kernel(**inputs):
    inputs = {k: np.asarray(v) for k, v in inputs.items()}
    nc = _get_program()
    res = run_bass_kernel_spmd(nc, _in_maps(inputs), list(range(B)))
    out = np.stack([res.results[i]["out"] for i in range(B)])
    return out.reshape(B, CIN, H, W).astype(np.float32)


def kernel_traced(**inputs):
    inputs = {k: np.asarray(v) for k, v in inputs.items()}
    nc = _get_program()
    res = run_bass_kernel_spmd(nc, _in_maps(inputs), list(range(B)), trace=True)
    out = np.stack([res.results[i]["out"] for i in range(B)])
    return out.reshape(B, CIN, H, W).astype(np.float32), res
